# Initial kernel scaffold
#
"""Trainium2 Bass kernel for elementwise i1e(z) = exp(-|z|) * I1(z),
z in [0.1, 50], shape (32, 4096, 1024) f32, data-parallel over 8 cores.

Algorithm (single branch, memory-regime friendly):
    t = z + B
    w = 1/sqrt(t)        (ACT: Ln then Exp(-0.5*ln))
    u = w*w = 1/t        (ACT: Square)
    v = ALPHA*u + BETA   (ACT: Copy with affine)  -> v in [-1, 1]
    p = Horner(deg-13 minimax poly in v)          (4 custom DVE insts)
    out = (p + c0) * w                            (1 custom DVE inst)

Max relative error vs scipy.special.i1e: ~5e-6 (dominated by f32 rounding).
"""
import numpy as np

NCORES = 8
NT, P, FD = 64, 128, 2048          # per-core: 64 tiles of [128, 2048] f32
FULL_SHAPE = (32, 4096, 1024)
PER_CORE = (4, 4096, 1024)

_DEG = 10          # 13 (6 DVE insts, ~1.3e-5) or 10 (5 insts, ~2e-5)

_B = 4.0
_ALPHA = 8.87374749498998
_BETA = -1.1643286573146292
# c0..c13, minimax-relative fit of i1e(x)*sqrt(x+B) in v = ALPHA/(x+B)+BETA
_MONO13 = [
    0.5113408798753526, 0.10100091287603755, -0.073057009643963,
    -0.1972353073824678, -0.22259214044271458, -0.10514275018682244,
    0.04594726025379211, 0.06290229842060668, -0.014911723943064734,
    -0.030026882056280426, 0.00635433329788149, 0.010390848736236805,
    -0.0014886726029969299, -0.0017597287076140495,
]
_B10 = 5.0
_ALPHA10 = 11.242484969939879
_BETA10 = -1.2044088176352705
_MONO10 = [
    0.529914476780326, 0.14219479767386273, -0.00779169209396241,
    -0.12747565698895386, -0.21429275664984493, -0.2048359623477035,
    -0.07890080267834916, 0.037884804968345835, 0.03708335892316636,
    -0.004248331255089073, -0.007234494171103434,
]
if _DEG == 10:
    _B, _ALPHA, _BETA, _MONO = _B10, _ALPHA10, _BETA10, _MONO10
else:
    _MONO = _MONO13

_state = {}


def _register_ops():
    import concourse.dve_ops as dve_ops
    from concourse.dve_spec import (
        Spec, Src0, Src1, C0, C1, C2, C3, _spill_c3_to_src1, lower,
        _has_src1, sq,
    )
    from concourse.dve_uop import DveOpSpec

    if "IVE_POLY4" in dve_ops._SUB_OPCODE_FOR_NAME:
        return {o.name: o for o in dve_ops.OPS}

    f32 = np.float32

    def ref_poly4(in0, in1, s0, s1, imm2):
        c3 = np.asarray(in1, f32).reshape(-1, 1)
        x = in0.astype(f32)
        return ((((s0 * x + s1) * x) + imm2) * x + c3).astype(f32)

    def ref_h3(in0, in1, s0, s1, imm2):
        p, v = in0.astype(f32), in1.astype(f32)
        return (((p * v + s0) * v + s1) * v + imm2).astype(f32)

    def ref_h3m(in0, in1, s0, s1, imm2):
        p, v = in0.astype(f32), in1.astype(f32)
        return ((((p * v + s0) * v + s1) * v + imm2) * v).astype(f32)

    def ref_final2(in0, in1, s0, s1, imm2):
        return ((in0.astype(f32) + s0) * in1.astype(f32)).astype(f32)

    specs = [
        ("IVE_POLY4", Spec(
            body=_spill_c3_to_src1(((C0 * Src0 + C1) * Src0 + C2) * Src0 + C3),
            reference=ref_poly4)),
        ("IVE_HORNER3", Spec(
            body=((Src0 * Src1 + C0) * Src1 + C1) * Src1 + C2,
            reference=ref_h3)),
        ("IVE_HORNER3M", Spec(
            body=(((Src0 * Src1 + C0) * Src1 + C1) * Src1 + C2) * Src1,
            reference=ref_h3m)),
        ("IVE_FINAL2", Spec(
            body=(Src0 + C0) * Src1,
            reference=ref_final2)),
        # v = C1 * (w^2*(C0 - t*w^2)) + C2 -- NR reciprocal cleanup of
        # u = w*w ~= 1/t (in1=w) against t (in0), fused with the affine
        # v-map.  u's seed error is squared; one NR gives ~1-2 ulp.
        ("IVE_NRVW", Spec(
            body=((C0 - Src0 * sq(Src1)) * sq(Src1)) * C1 + C2,
            reference=lambda in0, in1, s0, s1, imm2: (
                ((s0 - in0.astype(np.float32) * in1 * in1) * (in1 * in1))
                * s1 + imm2
            ).astype(np.float32))),
        # p = ((p*v + C0)*v + C1)*v  -- two Horner steps + trailing mul
        ("IVE_H2M", Spec(
            body=((Src0 * Src1 + C0) * Src1 + C1) * Src1,
            reference=lambda in0, in1, s0, s1, imm2: (
                ((in0.astype(np.float32) * in1 + s0) * in1 + s1) * in1
            ).astype(np.float32))),
    ]
    new_ops = []
    for name, spec in specs:
        op = dve_ops.DveOp(name, spec, subdim=False, uops_sha={})
        dve_ops.OPS.append(op)
        new_ops.append(op)
    dve_ops._SUB_OPCODE_FOR_NAME.update(
        {op.name: dve_ops._CUSTOM_DVE_ROW_BASE + i
         for i, op in enumerate(dve_ops.OPS)}
    )
    dve_ops.CUSTOM_DVE_SPECS.update({op.name: op.spec for op in new_ops})
    for op in new_ops:
        shas = {}
        for ver in ("v3", "v4"):
            try:
                s = DveOpSpec(
                    name=op.name,
                    opcode=dve_ops.get_dve_sub_opcode(op.name),
                    uops=lower(op.spec, ver=ver),
                    rd1_en=_has_src1(op.spec),
                )
                shas[ver] = s.sha(ver)
            except Exception:
                pass
        object.__setattr__(op, "uops_sha", shas)
    return {o.name: o for o in dve_ops.OPS}


def _make_pools(tc, ctx):
    pools = {}
    for name, bufs in [("x", 2), ("t", 2), ("a", 2), ("w", 2), ("v", 2),
                       ("p", 2), ("o", 3)]:
        pools[name] = ctx.enter_context(tc.tile_pool(name=name, bufs=bufs))
    return pools


def _emit_consts(nc, tc, ctx):
    from concourse import mybir
    F32 = mybir.dt.float32
    cpool = ctx.enter_context(tc.tile_pool(name="const", bufs=1))
    bias_b = cpool.tile([P, 1], F32, tag="bias_b")
    nc.vector.memset(bias_b[:], _B)
    bias_0 = cpool.tile([P, 1], F32, tag="bias_0")
    nc.vector.memset(bias_0[:], 0.0)
    ctail = cpool.tile([P, 1], F32, tag="ctail")
    nc.vector.memset(ctail[:], float(np.float32(_MONO[_DEG - 3])))
    return {"bias_b": bias_b, "bias_0": bias_0, "ctail": ctail}


def _emit_tile(nc, ops, pools, consts, c, src_ap, dst_ap):
    """One tile: DMA in -> ACT(t, ln, w) -> DVE(v, poly, out) -> DMA out."""
    from concourse import mybir
    F32 = mybir.dt.float32
    AF = mybir.ActivationFunctionType
    H3, F2, NRVW, H2M = (ops["IVE_HORNER3"], ops["IVE_FINAL2"],
                         ops["IVE_NRVW"], ops["IVE_H2M"])

    POLY4, H3M = ops["IVE_POLY4"], ops["IVE_HORNER3M"]
    D = _DEG

    xt = pools["x"].tile([P, FD], F32, tag="x")
    nc.sync.dma_start(out=xt[:], in_=src_ap)
    tt = pools["t"].tile([P, FD], F32, tag="t")
    nc.scalar.activation(tt[:], xt[:], AF.Copy, bias=_B, scale=1.0)
    at = pools["a"].tile([P, FD], F32, tag="a")
    nc.scalar.activation(at[:], xt[:], AF.Ln, bias=consts["bias_b"][:],
                         scale=1.0)
    wt = pools["w"].tile([P, FD], F32, tag="w")
    nc.scalar.activation(wt[:], at[:], AF.Exp, bias=consts["bias_0"][:],
                         scale=-0.5)
    vt = pools["v"].tile([P, FD], F32, tag="v")
    nc.vector._custom_dve(NRVW, out=vt[:], in0=tt[:], in1=wt[:],
                          s0=2.0, s1=_ALPHA, imm2=_BETA)
    # POLY4: p = ((c[D]*v + c[D-1])*v + c[D-2])*v + c[D-3]   (c[D-3] via in1)
    p = pools["p"].tile([P, FD], F32, tag="pA")
    nc.vector._custom_dve(POLY4, out=p[:], in0=vt[:], in1=consts["ctail"][:],
                          s0=c[D], s1=c[D - 1], imm2=c[D - 2])
    # H3 x a: 3 coeffs each, down to c[4]
    k = D - 4
    tags = ["pB", "pC", "pD"]
    ti = 0
    while k >= 6:
        pn = pools["p"].tile([P, FD], F32, tag=tags[ti]); ti += 1
        nc.vector._custom_dve(H3, out=pn[:], in0=p[:], in1=vt[:],
                              s0=c[k], s1=c[k - 1], imm2=c[k - 2])
        p = pn
        k -= 3
    assert k == 3, k
    # H3M: c3, c2, c1 + trailing *v
    pm = pools["p"].tile([P, FD], F32, tag="pE")
    nc.vector._custom_dve(H3M, out=pm[:], in0=p[:], in1=vt[:],
                          s0=c[3], s1=c[2], imm2=c[1])
    ot = pools["o"].tile([P, FD], F32, tag="o")
    nc.vector._custom_dve(F2, out=ot[:], in0=pm[:], in1=wt[:], s0=c[0])
    nc.sync.dma_start(out=dst_ap, in_=ot[:])


def _build_nc():
    import concourse.bacc as bacc
    import concourse.tile as tile
    from concourse import mybir
    from contextlib import ExitStack

    ops = _register_ops()
    c = [float(np.float32(q)) for q in _MONO]
    F32 = mybir.dt.float32

    nc = bacc.Bacc(
        "TRN2", target_bir_lowering=False, debug=False,
        enable_asserts=True, num_devices=NCORES,
    )
    z = nc.dram_tensor("z", [NT, P, FD], F32, kind="ExternalInput").ap()
    out = nc.dram_tensor("out", [NT, P, FD], F32, kind="ExternalOutput").ap()

    with tile.TileContext(nc) as tc, ExitStack() as ctx:
        consts = _emit_consts(nc, tc, ctx)
        pools = _make_pools(tc, ctx)
        for i in range(NT):
            _emit_tile(nc, ops, pools, consts, c, z[i], out[i])
    nc.compile()
    return nc


def _get_nc():
    if "nc" not in _state:
        _state["nc"] = _build_nc()
    return _state["nc"]


def kernel(z: np.ndarray) -> np.ndarray:
    from concourse.bass_utils import run_bass_kernel_spmd

    z = np.ascontiguousarray(z, dtype=np.float32)
    assert z.shape == FULL_SHAPE, z.shape
    nc = _get_nc()
    shards = z.reshape(NCORES, NT, P, FD)
    in_maps = [{"z": shards[i]} for i in range(NCORES)]
    try:
        res = run_bass_kernel_spmd(nc, in_maps, list(range(NCORES)))
    except Exception:
        res = run_bass_kernel_spmd(nc, in_maps, list(range(NCORES)))
    outs = [res.results[i]["out"].reshape(PER_CORE) for i in range(NCORES)]
    return np.concatenate(outs, axis=0)



# revision 1
# speedup vs baseline: 1.5465x; 1.5465x over previous
"""Trainium2 Bass kernel for elementwise i1e(z) = exp(-|z|) * I1(z),
z in [0.1, 50], shape (32, 4096, 1024) f32, data-parallel over 8 cores.

Algorithm (single branch, memory-regime friendly):
    t = z + B
    w = 1/sqrt(t)        (ACT: Ln then Exp(-0.5*ln))
    u = w*w = 1/t        (ACT: Square)
    v = ALPHA*u + BETA   (ACT: Copy with affine)  -> v in [-1, 1]
    p = Horner(deg-13 minimax poly in v)          (4 custom DVE insts)
    out = (p + c0) * w                            (1 custom DVE inst)

Max relative error vs scipy.special.i1e: ~5e-6 (dominated by f32 rounding).
"""
import numpy as np

NCORES = 8
NT, P, FD = 64, 128, 2048          # per-core: 64 tiles of [128, 2048] f32
FULL_SHAPE = (32, 4096, 1024)
PER_CORE = (4, 4096, 1024)

_DEG = 10          # 13 (6 DVE insts, ~1.3e-5) or 10 (5 insts, ~2e-5)

_B = 4.0
_ALPHA = 8.87374749498998
_BETA = -1.1643286573146292
# c0..c13, minimax-relative fit of i1e(x)*sqrt(x+B) in v = ALPHA/(x+B)+BETA
_MONO13 = [
    0.5113408798753526, 0.10100091287603755, -0.073057009643963,
    -0.1972353073824678, -0.22259214044271458, -0.10514275018682244,
    0.04594726025379211, 0.06290229842060668, -0.014911723943064734,
    -0.030026882056280426, 0.00635433329788149, 0.010390848736236805,
    -0.0014886726029969299, -0.0017597287076140495,
]
_B10 = 5.0
_ALPHA10 = 11.242484969939879
_BETA10 = -1.2044088176352705
_MONO10 = [
    0.529914476780326, 0.14219479767386273, -0.00779169209396241,
    -0.12747565698895386, -0.21429275664984493, -0.2048359623477035,
    -0.07890080267834916, 0.037884804968345835, 0.03708335892316636,
    -0.004248331255089073, -0.007234494171103434,
]
if _DEG == 10:
    _B, _ALPHA, _BETA, _MONO = _B10, _ALPHA10, _BETA10, _MONO10
else:
    _MONO = _MONO13

_state = {}


def _register_ops():
    import concourse.dve_ops as dve_ops
    from concourse.dve_spec import (
        Spec, Src0, Src1, C0, C1, C2, C3, _spill_c3_to_src1, lower,
        _has_src1, sq,
    )
    from concourse.dve_uop import DveOpSpec

    if "IVE_POLY4" in dve_ops._SUB_OPCODE_FOR_NAME:
        return {o.name: o for o in dve_ops.OPS}

    f32 = np.float32

    def ref_poly4(in0, in1, s0, s1, imm2):
        c3 = np.asarray(in1, f32).reshape(-1, 1)
        x = in0.astype(f32)
        return ((((s0 * x + s1) * x) + imm2) * x + c3).astype(f32)

    def ref_h3(in0, in1, s0, s1, imm2):
        p, v = in0.astype(f32), in1.astype(f32)
        return (((p * v + s0) * v + s1) * v + imm2).astype(f32)

    def ref_h3m(in0, in1, s0, s1, imm2):
        p, v = in0.astype(f32), in1.astype(f32)
        return ((((p * v + s0) * v + s1) * v + imm2) * v).astype(f32)

    def ref_final2(in0, in1, s0, s1, imm2):
        return ((in0.astype(f32) + s0) * in1.astype(f32)).astype(f32)

    specs = [
        ("IVE_POLY4", Spec(
            body=_spill_c3_to_src1(((C0 * Src0 + C1) * Src0 + C2) * Src0 + C3),
            reference=ref_poly4)),
        ("IVE_HORNER3", Spec(
            body=((Src0 * Src1 + C0) * Src1 + C1) * Src1 + C2,
            reference=ref_h3)),
        ("IVE_HORNER3M", Spec(
            body=(((Src0 * Src1 + C0) * Src1 + C1) * Src1 + C2) * Src1,
            reference=ref_h3m)),
        ("IVE_FINAL2", Spec(
            body=(Src0 + C0) * Src1,
            reference=ref_final2)),
        # v = C1 * (w^2*(C0 - t*w^2)) + C2 -- NR reciprocal cleanup of
        # u = w*w ~= 1/t (in1=w) against t (in0), fused with the affine
        # v-map.  u's seed error is squared; one NR gives ~1-2 ulp.
        ("IVE_NRVW", Spec(
            body=((C0 - Src0 * sq(Src1)) * sq(Src1)) * C1 + C2,
            reference=lambda in0, in1, s0, s1, imm2: (
                ((s0 - in0.astype(np.float32) * in1 * in1) * (in1 * in1))
                * s1 + imm2
            ).astype(np.float32))),
        # p = ((p*v + C0)*v + C1)*v  -- two Horner steps + trailing mul
        ("IVE_H2M", Spec(
            body=((Src0 * Src1 + C0) * Src1 + C1) * Src1,
            reference=lambda in0, in1, s0, s1, imm2: (
                ((in0.astype(np.float32) * in1 + s0) * in1 + s1) * in1
            ).astype(np.float32))),
    ]
    new_ops = []
    for name, spec in specs:
        op = dve_ops.DveOp(name, spec, subdim=False, uops_sha={})
        dve_ops.OPS.append(op)
        new_ops.append(op)
    dve_ops._SUB_OPCODE_FOR_NAME.update(
        {op.name: dve_ops._CUSTOM_DVE_ROW_BASE + i
         for i, op in enumerate(dve_ops.OPS)}
    )
    dve_ops.CUSTOM_DVE_SPECS.update({op.name: op.spec for op in new_ops})
    for op in new_ops:
        shas = {}
        for ver in ("v3", "v4"):
            try:
                s = DveOpSpec(
                    name=op.name,
                    opcode=dve_ops.get_dve_sub_opcode(op.name),
                    uops=lower(op.spec, ver=ver),
                    rd1_en=_has_src1(op.spec),
                )
                shas[ver] = s.sha(ver)
            except Exception:
                pass
        object.__setattr__(op, "uops_sha", shas)
    return {o.name: o for o in dve_ops.OPS}


def _make_pools(tc, ctx):
    pools = {}
    for name, bufs in [("x", 2), ("t", 2), ("a", 2), ("w", 2), ("v", 2),
                       ("p", 2), ("o", 3)]:
        pools[name] = ctx.enter_context(tc.tile_pool(name=name, bufs=bufs))
    return pools


def _emit_consts(nc, tc, ctx):
    from concourse import mybir
    F32 = mybir.dt.float32
    cpool = ctx.enter_context(tc.tile_pool(name="const", bufs=1))
    bias_b = cpool.tile([P, 1], F32, tag="bias_b")
    nc.vector.memset(bias_b[:], _B)
    bias_0 = cpool.tile([P, 1], F32, tag="bias_0")
    nc.vector.memset(bias_0[:], 0.0)
    ctail = cpool.tile([P, 1], F32, tag="ctail")
    nc.vector.memset(ctail[:], float(np.float32(_MONO[_DEG - 3])))
    return {"bias_b": bias_b, "bias_0": bias_0, "ctail": ctail}


def _emit_tile(nc, ops, pools, consts, c, src_ap, dst_ap):
    """One tile: DMA in -> ACT(t, ln, w) -> DVE(v, poly, out) -> DMA out."""
    from concourse import mybir
    F32 = mybir.dt.float32
    AF = mybir.ActivationFunctionType
    H3, F2, NRVW, H2M = (ops["IVE_HORNER3"], ops["IVE_FINAL2"],
                         ops["IVE_NRVW"], ops["IVE_H2M"])

    POLY4, H3M = ops["IVE_POLY4"], ops["IVE_HORNER3M"]
    D = _DEG

    xt = pools["x"].tile([P, FD], F32, tag="x")
    nc.sync.dma_start(out=xt[:], in_=src_ap)
    tt = pools["t"].tile([P, FD], F32, tag="t")
    nc.scalar.activation(tt[:], xt[:], AF.Copy, bias=_B, scale=1.0)
    at = pools["a"].tile([P, FD], F32, tag="a")
    nc.scalar.activation(at[:], xt[:], AF.Ln, bias=consts["bias_b"][:],
                         scale=1.0)
    wt = pools["w"].tile([P, FD], F32, tag="w")
    nc.scalar.activation(wt[:], at[:], AF.Exp, bias=consts["bias_0"][:],
                         scale=-0.5)
    vt = pools["v"].tile([P, FD], F32, tag="v")
    nc.vector._custom_dve(NRVW, out=vt[:], in0=tt[:], in1=wt[:],
                          s0=2.0, s1=_ALPHA, imm2=_BETA)
    # POLY4: p = ((c[D]*v + c[D-1])*v + c[D-2])*v + c[D-3]   (c[D-3] via in1)
    p = pools["p"].tile([P, FD], F32, tag="pA")
    nc.vector._custom_dve(POLY4, out=p[:], in0=vt[:], in1=consts["ctail"][:],
                          s0=c[D], s1=c[D - 1], imm2=c[D - 2])
    # H3 x a: 3 coeffs each, down to c[4]
    k = D - 4
    tags = ["pB", "pC", "pD"]
    ti = 0
    while k >= 6:
        pn = pools["p"].tile([P, FD], F32, tag=tags[ti]); ti += 1
        nc.vector._custom_dve(H3, out=pn[:], in0=p[:], in1=vt[:],
                              s0=c[k], s1=c[k - 1], imm2=c[k - 2])
        p = pn
        k -= 3
    assert k == 3, k
    # H3M: c3, c2, c1 + trailing *v
    pm = pools["p"].tile([P, FD], F32, tag="pE")
    nc.vector._custom_dve(H3M, out=pm[:], in0=p[:], in1=vt[:],
                          s0=c[3], s1=c[2], imm2=c[1])
    ot = pools["o"].tile([P, FD], F32, tag="o")
    nc.vector._custom_dve(F2, out=ot[:], in0=pm[:], in1=wt[:], s0=c[0])
    nc.sync.dma_start(out=dst_ap, in_=ot[:])


def _build_nc():
    import concourse.bacc as bacc
    import concourse.tile as tile
    from concourse import mybir
    from contextlib import ExitStack

    ops = _register_ops()
    c = [float(np.float32(q)) for q in _MONO]
    F32 = mybir.dt.float32

    nc = bacc.Bacc(
        "TRN2", target_bir_lowering=False, debug=False,
        enable_asserts=True, num_devices=NCORES,
    )
    z = nc.dram_tensor("z", [NT, P, FD], F32, kind="ExternalInput").ap()
    out = nc.dram_tensor("out", [NT, P, FD], F32, kind="ExternalOutput").ap()

    with tile.TileContext(nc) as tc, ExitStack() as ctx:
        consts = _emit_consts(nc, tc, ctx)
        pools = _make_pools(tc, ctx)
        for i in range(NT):
            _emit_tile(nc, ops, pools, consts, c, z[i], out[i])
    nc.compile()
    return nc


def _get_nc():
    if "nc" not in _state:
        _state["nc"] = _build_nc()
    return _state["nc"]


def kernel(z: np.ndarray) -> np.ndarray:
    from concourse.bass_utils import run_bass_kernel_spmd

    z = np.ascontiguousarray(z, dtype=np.float32)
    assert z.shape == FULL_SHAPE, z.shape
    nc = _get_nc()
    shards = z.reshape(NCORES, NT, P, FD)
    in_maps = [{"z": shards[i]} for i in range(NCORES)]
    try:
        res = run_bass_kernel_spmd(nc, in_maps, list(range(NCORES)))
    except Exception:
        res = run_bass_kernel_spmd(nc, in_maps, list(range(NCORES)))
    outs = [res.results[i]["out"].reshape(PER_CORE) for i in range(NCORES)]
    return np.concatenate(outs, axis=0)

